# revision 79
# baseline (speedup 1.0000x reference)
"""DeepSetPred Trainium2 kernel: 3-layer token encoder MLP + segment-sum +
predictor MLP on 8 NeuronCores, ZERO collectives.

Key structural wins over the naive mapping:

1. The third encoder layer is linear, so it commutes with the segment-sum:
       enc = segsum(h2 @ W3 + b3) = segsum(h2) @ W3 + counts * b3
   The per-token L3 matmul (25% of encoder FLOPs) and the one-hot
   segment-matmul collapse into one tiny [S,H]x[H,C] matmul after pooling.

2. segsum(h2) is computed without materializing anything token-major:
   - The host lays each core's (sorted) tokens out so every chunk contains
     at most ONE segment transition, inserting a few zero pad tokens when
     two boundaries would share a chunk.  One pad token is prepended so the
     pad vector v = tanh(W2'tanh(b1)+b2) can be captured on-device from
     chunk 0 (v corrections for pad-polluted sums are exact).
   - The L2 tanh on the ScalarEngine emits accum_out = full-chunk sum per
     H-partition (free side output).
   - One DVE scalar_tensor_tensor per (h-tile, chunk) computes the head
     sum via the fused (iota is_lt b) mult h2 -> accum_out form, with b
     per chunk as DATA, so the instruction stream is identical across all
     8 cores -> one NEFF.
   - hsegT[h, s] = sum_c head_c*XA[c,s] + full_c*XB[c,s] - npad*v, with
     the coefficient stack X as per-core input data, evaluated by tiny PE
     matmuls after transposing the accumulator.  The transpose+assembly
     runs in 3 stages; the first two are issued mid-loop (hidden under
     the token chunks), only the last 2 chunk columns assemble in the
     epilogue.

3. Everything stays fp16 on the matmul path (fp8 fails here: the ragged
   pooling amplifies per-token quantization noise by sqrt(count) ~ 32x).

4. Startup: the first chunks are small (512 tokens) and the startup-
   critical tensors lead 4 independent DMA rings (sync/gpsimd/scalar/
   vector) so the PE starts ~10us in (DMA pipe latency floor) and never
   starves.  Weights are packed into a few large blobs to cut DMA queue
   count (queue drains dominate NEFF teardown time).

5. Epilogue: predictor biases are applied via rank-1 PE matmuls so each
   tanh layer is ONE wide ACT over all 4 h-tiles; the whole predictor is
   a short pipelined chain of tiny matmuls.

Sharding: host cuts the sorted token axis at segment boundaries so every
segment belongs to exactly one core (no collectives); each core runs the
predictor on its own <=SLOTS segments and writes its slice of the output.
"""

import bisect

import numpy as np

import concourse.mybir as mybir
import concourse.tile as tile
from concourse import bacc
from concourse import bass_utils
from concourse.masks import make_identity

# Problem shapes (hardcoded per contract).
T, E, H, C, O = 131072, 256, 512, 256, 32
S = 128            # num segments
N_CORES = 8
TOK = 1024         # steady-state tokens per chunk
MIN_SLOTS = 32     # segments-per-core capacity
S2 = 16            # slots reserved for "late" segments (touch stage-C
                   # chunks); their predictor runs in the epilogue,
                   # everyone else's overlaps the token loop
F32 = mybir.dt.float32
F16 = mybir.dt.float16

EC = E // 128      # 2
HC = H // 128      # 4
CC = C // 128      # 2

_CACHE = {}
_LAST_IN_MAPS = None


def make_schedule(t_sh):
    """Chunk sizes: two 512s to fill the DMA pipeline fast, 1024s in the
    steady state, and a small-chunk tail so the final ACT/STT drain is
    short.  Only ever SUBDIVIDES the 512/512/1024... grid, so the host
    layout's <=1-transition-per-grid-interval guarantee carries over."""
    sched = []
    rem = t_sh
    for s in (512, 512):
        if rem <= 0:
            break
        c = min(s, rem)
        sched.append(c)
        rem -= c
    while rem > 1152:
        sched.append(TOK)
        rem -= TOK
    while rem > 640:
        sched.append(512)
        rem -= 512
    if rem > 128:
        sched.append(rem - 128)
        rem = 128
    if rem > 0:
        sched.append(rem)
    return sched


def chunk_ends(upto):
    """Ascending chunk END positions of the (t_sh-independent) grid."""
    ends = [512, 1024]
    while ends[-1] < upto:
        ends.append(ends[-1] + TOK)
    return ends


def make_groups(NC):
    """xt DMA groups: 4 single chunks then pairs (fewer DMA queues)."""
    g = []
    i = 0
    while i < NC:
        n = 1 if i < 4 else min(2, NC - i)
        g.append((i, n))
        i += n
    return g


def _mm(nc, out, lhsT, rhs, start, stop):
    nc.tensor.matmul(out, lhsT, rhs,
                     start=start, stop=stop, skip_group_check=True)


def _build_nc(t_sh, SLOTS):
    assert t_sh % 128 == 0
    schedule = make_schedule(t_sh)
    NC = len(schedule)
    CA = 9                       # stage-A accumulator columns (v + 8 chunks)
    assert NC >= CA + 3, NC
    NCOL = NC + 1                # acc columns: 0 = v, 1..NC = chunks
    CB = NC - 1                  # stage C covers the last two chunks
    RA, RB, RC = 2 * CA, 2 * (CB - CA), 2 * (NC + 1 - CB)
    assert CA < CB < NC and RA <= 32 and RB <= 32 and RC <= 32
    RT = 64 + RC                 # accT rows; stages at partitions 0/32/64
    # rowblob column offsets (must match host); all operands live in ONE
    # SBUF row so every matmul slice has base partition 0
    RBM3 = 0
    RPB3 = RBM3 + H
    RCNT = RPB3 + O
    RPB1 = RCNT + SLOTS
    RPB2 = RPB1 + H
    RBCOLS = RPB2 + H

    nc = bacc.Bacc("TRN2", target_bir_lowering=False, debug=False,
                   num_devices=N_CORES)

    xt_d = nc.dram_tensor("xt", [128, EC * t_sh], F16, kind="ExternalInput")
    w1h0_d = nc.dram_tensor("w1h0", [128, EC * 128], F16,
                            kind="ExternalInput")
    w1r_d = nc.dram_tensor("w1r", [128, EC * (HC - 1) * 128], F16,
                           kind="ExternalInput")
    w2a_d = nc.dram_tensor("w2a", [128, 2 * HC * 128], F16,
                           kind="ExternalInput")
    w2b_d = nc.dram_tensor("w2b", [128, 2 * HC * 128], F16,
                           kind="ExternalInput")
    # blob16: [M=W3@P1 (fused, skips the enc stage) | p2 | p3] layout
    B16M = 0
    B16P2 = B16M + HC * HC * 128
    B16P3 = B16P2 + HC * HC * 128
    B16COLS = B16P3 + HC * O
    blob16_d = nc.dram_tensor("blob16", [128, B16COLS], F16,
                              kind="ExternalInput")
    # blob32: [b1 (HC) | b2 (HC) | bm1 (NC)]
    blob32_d = nc.dram_tensor("blob32", [128, 2 * HC + NC], F32,
                              kind="ExternalInput")
    cfb_d = nc.dram_tensor("cfb", [RT, SLOTS], F16, kind="ExternalInput")
    rowb_d = nc.dram_tensor("rowb", [1, RBCOLS], F16, kind="ExternalInput")
    iota_d = nc.dram_tensor("iota", [128, TOK], F16, kind="ExternalInput")
    out_d = nc.dram_tensor("pred", [SLOTS, O], F32, kind="ExternalOutput")

    LT = mybir.AluOpType.is_lt
    MUL = mybir.AluOpType.mult
    TANH = mybir.ActivationFunctionType.Tanh

    groups = make_groups(NC)
    chunk_group = {}
    starts = np.concatenate([[0], np.cumsum(schedule)]).astype(int)
    for gi, (c0, n) in enumerate(groups):
        for k in range(n):
            ci = c0 + k
            chunk_group[ci] = (gi, int(starts[ci] - starts[c0]))

    with tile.TileContext(nc) as tc:
        with tc.tile_pool(name="wts", bufs=1) as wp, \
             tc.tile_pool(name="xt", bufs=4) as xtp, \
             tc.tile_pool(name="h1", bufs=2) as h1p, \
             tc.tile_pool(name="h2", bufs=3) as h2p, \
             tc.tile_pool(name="scr", bufs=3) as scp, \
             tc.tile_pool(name="small", bufs=1) as smp, \
             tc.tile_pool(name="ps1", bufs=2, space="PSUM") as ps1p, \
             tc.tile_pool(name="ps2", bufs=2, space="PSUM") as ps2p:

            # warm the ACT tanh table before the scalar queue fills
            warm_sb = smp.tile([1, 1], F32, tag="warm", name="warm")
            nc.gpsimd.memset(warm_sb[:], 0.0)
            warm_o = smp.tile([1, 1], F32, tag="warmo", name="warmo")
            nc.scalar.activation(warm_o[:], warm_sb[:], TANH)

            # ---- startup-critical DMAs on 3 rings, in priority order.
            # Chunk 0 rides the sync ring ALONE so the first matmuls gate
            # on exactly two queues (sync: c0, scalar: w1h0); weights are
            # split scalar (w1) / gpsimd (biases + w2 + iota) so each
            # queue's serial delivery matches when the pipeline needs it.
            w1h0_t = wp.tile([128, EC, 128], F16, tag="w1h0", name="w1h0")
            nc.scalar.dma_start(
                w1h0_t[:], w1h0_d.ap().rearrange("p (e q) -> p e q", e=EC))
            w1r_t = wp.tile([128, EC, HC - 1, 128], F16, tag="w1r",
                            name="w1r")
            w1r_src = w1r_d.ap().rearrange("p (e h q) -> p e h q",
                                           e=EC, q=128)
            # h1's tile first so L1(h1) isn't gated on all of w1r
            nc.scalar.dma_start(w1r_t[:, :, 0:1, :], w1r_src[:, :, 0:1, :])
            nc.scalar.dma_start(w1r_t[:, :, 1:, :], w1r_src[:, :, 1:, :])
            cfb_sb = smp.tile([RT, SLOTS], F16, tag="cfb", name="cfb")
            nc.scalar.dma_start(cfb_sb[:], cfb_d.ap())
            rowb_sb = smp.tile([1, RBCOLS], F16, tag="rowb", name="rowb")
            nc.scalar.dma_start(rowb_sb[:], rowb_d.ap())
            blob32_sb = smp.tile([128, 2 * HC + NC], F32, tag="b32",
                                 name="b32")
            nc.gpsimd.dma_start(blob32_sb[:], blob32_d.ap())
            w2a_t = wp.tile([128, 2, HC, 128], F16, tag="w2a", name="w2a")
            nc.gpsimd.dma_start(
                w2a_t[:], w2a_d.ap().rearrange("p (k h q) -> p k h q",
                                               k=2, q=128))
            w2b_t = wp.tile([128, 2, HC, 128], F16, tag="w2b", name="w2b")
            nc.gpsimd.dma_start(
                w2b_t[:],
                w2b_d.ap().rearrange("p (k h q) -> p k h q", k=2, q=128))
            iota_sb = smp.tile([128, TOK], F16, tag="iota", name="iota")
            nc.gpsimd.dma_start(iota_sb[:], iota_d.ap())
            # epilogue blob: not needed until ~190us; held back so it
            # cannot compete with the startup crunch for HBM bandwidth
            w3p_t = wp.tile([128, B16COLS], F16, tag="b16", name="b16")

            w1_sb = [[(w1h0_t[:, e, :] if h == 0 else w1r_t[:, e, h - 1, :])
                      for h in range(HC)] for e in range(EC)]
            w2_sb = [[(w2a_t[:, k, h, :] if k < 2 else w2b_t[:, k - 2, h, :])
                      for h in range(HC)] for k in range(HC)]
            m_sb = [[w3p_t[:, B16M + (k * HC + h) * 128:
                            B16M + (k * HC + h + 1) * 128]
                      for h in range(HC)] for k in range(HC)]
            p2_sb = [[w3p_t[:, B16P2 + (k * HC + h) * 128:
                            B16P2 + (k * HC + h + 1) * 128]
                      for h in range(HC)] for k in range(HC)]
            p3_sb = [w3p_t[:, B16P3 + k * O:B16P3 + (k + 1) * O]
                     for k in range(HC)]
            b1_sb = blob32_sb[:, 0:HC]
            b2_sb = blob32_sb[:, HC:2 * HC]
            bm1_sb = blob32_sb[:, 2 * HC:2 * HC + NC]

            # ---- xt group tiles: issue every group DMA up front on the
            # sync ring; pool semaphores throttle execution ----
            gmax = max(int(starts[c0 + n] - starts[c0])
                       for (c0, n) in groups)
            gtiles = []
            for gi, (c0, n) in enumerate(groups):
                glen = int(starts[c0 + n] - starts[c0])
                gt = xtp.tile([128, EC, glen], F16, tag="xt", name="xt",
                              padded_shape=[128, EC, gmax])
                src = xt_d.ap().rearrange("p (e t) -> p e t", e=EC)
                gb = int(starts[c0])
                # stagger the early big-chunk fetches so they don't steal
                # HBM bandwidth from the critical weight loads
                delay = {2: 0.0115, 3: 0.013}.get(gi)
                with tc.tile_wait_until(delay, enable=delay is not None):
                    if gi in (0, 1, 2):
                        # early chunks in halves: their first matmul
                        # pieces start ~1us sooner, keeping the PE clock
                        # ramp unbroken through the startup crunch
                        half = glen // 2
                        nc.sync.dma_start(gt[:, :, 0:half],
                                          src[:, :, gb:gb + half])
                        nc.sync.dma_start(gt[:, :, half:glen],
                                          src[:, :, gb + half:gb + glen])
                    else:
                        nc.sync.dma_start(gt[:], src[:, :, gb:gb + glen])
                if gi == 4:
                    with tc.tile_wait_until(0.025):
                        nc.sync.dma_start(w3p_t[:], blob16_d.ap())
                gtiles.append(gt)

            ones1 = smp.tile([1, SLOTS], F16, tag="ones1", name="ones1")
            nc.gpsimd.memset(ones1[:], 1.0)
            ident = smp.tile([128, 128], F32, tag="ident", name="ident")
            make_identity(nc, ident[:])

            # pooling accumulators: [h, col, plane]; col 0 plane 0 = v,
            # cols 1..NC = chunks; plane 0 = head sum, plane 1 = full sum
            acc_all = smp.tile([128, HC, NCOL, 2], F32, tag="acc",
                               name="acc")
            nc.gpsimd.memset(acc_all[:], 0.0)

            def pieces(ci, ct):
                if ci in (0, 1) and ct == 512:
                    return [(0, 256), (256, 256)]
                if ct <= 512:
                    return [(0, ct)]
                return [(0, 512), (512, ct - 512)]

            def load_chunk(ci):
                ct = schedule[ci]
                gi, off = chunk_group[ci]
                h1_t = h1p.tile([128, HC, ct], F16, tag="h1", name="h1",
                                padded_shape=[128, HC, TOK])
                h2_t = h2p.tile([128, HC, ct], F16, tag="h2", name="h2",
                                padded_shape=[128, HC, TOK])
                return (ci, ct, gtiles[gi], off, h1_t, h2_t)

            def l1_h(st, h):
                ci, ct, xt_t, off, h1_t, _ = st
                ps1 = ps1p.tile([128, ct], F32, tag="ps1", name="ps1",
                                padded_shape=[128, TOK])
                for (hb, hl) in pieces(ci, ct):
                    for e in range(EC):
                        _mm(nc, ps1[:, hb:hb + hl], w1_sb[e][h],
                            xt_t[:, e, off + hb:off + hb + hl],
                            start=(e == 0), stop=(e == EC - 1))
                nc.scalar.activation(h1_t[:, h, :], ps1[:], TANH,
                                     bias=b1_sb[:, h:h + 1])

            def l2_h(st, h):
                ci, ct, _, _, h1_t, h2_t = st
                # final chunk: no L1 work remains, so borrow ps1p for the
                # odd h-groups -- all 4 L2 groups get their own PSUM buffer
                # and drain through ACT/STT in parallel instead of
                # serializing on ps2p's two buffers
                if ci == NC - 1 and h % 2 == 1:
                    ps2 = ps1p.tile([128, ct], F32, tag="ps1", name="ps2f",
                                    padded_shape=[128, TOK])
                else:
                    ps2 = ps2p.tile([128, ct], F32, tag="ps2", name="ps2",
                                    padded_shape=[128, TOK])
                for (hb, hl) in pieces(ci, ct):
                    for k in range(HC):
                        _mm(nc, ps2[:, hb:hb + hl], w2_sb[k][h],
                            h1_t[:, k, hb:hb + hl],
                            start=(k == 0), stop=(k == HC - 1))
                nc.scalar.activation(h2_t[:, h, :], ps2[:], TANH,
                                     bias=b2_sb[:, h:h + 1],
                                     accum_out=acc_all[:, h, ci + 1, 1:2])
                # head sum: (iota < b) * h2 summed along tokens, b is data
                sc = scp.tile([128, ct], F16, tag="scr", name="scr",
                              padded_shape=[128, TOK])
                nc.vector.scalar_tensor_tensor(
                    sc[:], iota_sb[:, :ct], bm1_sb[:, ci:ci + 1],
                    h2_t[:, h, :], LT, MUL,
                    accum_out=acc_all[:, h, ci + 1, 0:1])

            # staged accumulator assembly: transpose columns as soon as
            # their chunks are done, overlapped with the token loop.  All
            # stages land in ONE stack (at partition offsets 0/32/64 --
            # engine accesses must start at a multiple of 32) so hsegT
            # needs one matmul per h; the gap rows multiply by zero
            # coefficients and are memset once so they can't be NaN.
            accT = smp.tile([RT, HC, 128], F16, tag="accT", name="accT")
            nc.gpsimd.memset(accT[:], 0.0)

            def round_tr(r0, rows, c0, c1):
                for h in range(HC):
                    tp = ps1p.tile([rows, 128], F32, tag="ps1", name="tp")
                    nc.tensor.transpose(tp[:], acc_all[:, h, c0:c1, :],
                                        ident[:])
                    nc.vector.tensor_copy(accT[r0:r0 + rows, h, :], tp[:])

            # ---- predictor MLP over a slot range; biases via rank-1
            # matmuls so each tanh layer is ONE wide ACT over all h tiles.
            # Runs twice: slots [0:S1) ("early" segments, no stage-C
            # coefficients) are issued right after round B so they overlap
            # the last token chunks; slots [S1:SLOTS) run in the epilogue.
            def predictor(s0, s1, tag):
                ns = s1 - s0
                hs_ps = ps2p.tile([128, HC, ns], F32, tag="ps2",
                                  name="hs" + tag)
                for h in range(HC):
                    _mm(nc, hs_ps[:, h, :], accT[:, h, :],
                        cfb_sb[:, s0:s1], start=True, stop=True)
                hsegT = smp.tile([128, HC, ns], F16, tag="hsegT" + tag,
                                 name="hsegT" + tag)
                nc.vector.tensor_copy(hsegT[:], hs_ps[:])

                # q1_pre = (W3 P1)^T hseg + (P1^T b3) x cnt + pb1 x 1
                # (W3@P1 fused on the host -> the enc stage disappears)
                q1_ps = ps2p.tile([128, HC, ns], F32, tag="ps2",
                                  name="q1p" + tag)
                for h in range(HC):
                    nc.tensor.matmul(q1_ps[:, h, :],
                                     rowb_sb[:, RBM3 + h * 128:
                                             RBM3 + (h + 1) * 128],
                                     rowb_sb[:, RCNT + s0:RCNT + s1],
                                     start=True, stop=False,
                                     skip_group_check=True)
                    nc.tensor.matmul(q1_ps[:, h, :],
                                     rowb_sb[:, RPB1 + h * 128:
                                             RPB1 + (h + 1) * 128],
                                     ones1[:, s0:s1], start=False,
                                     stop=False, skip_group_check=True)
                    for k in range(HC):
                        _mm(nc, q1_ps[:, h, :], m_sb[k][h],
                            hsegT[:, k, :], start=False,
                            stop=(k == HC - 1))
                q1_sb = smp.tile([128, HC, ns], F16, tag="q1" + tag,
                                 name="q1" + tag)
                nc.scalar.activation(q1_sb[:], q1_ps[:], TANH)

                q2_ps = ps1p.tile([128, HC, ns], F32, tag="ps1",
                                  name="q2p" + tag)
                for h in range(HC):
                    nc.tensor.matmul(q2_ps[:, h, :],
                                     rowb_sb[:, RPB2 + h * 128:
                                             RPB2 + (h + 1) * 128],
                                     ones1[:, s0:s1], start=True,
                                     stop=False, skip_group_check=True)
                    for k in range(HC):
                        _mm(nc, q2_ps[:, h, :], p2_sb[k][h],
                            q1_sb[:, k, :], start=False,
                            stop=(k == HC - 1))
                q2_sb = smp.tile([128, HC, ns], F16, tag="q2" + tag,
                                 name="q2" + tag)
                nc.scalar.activation(q2_sb[:], q2_ps[:], TANH)

                # final: pred[slot, o] = q2.T @ P3 + pb3
                ppo = ps2p.tile([ns, O], F32, tag="ps2", name="ppo" + tag)
                nc.tensor.matmul(ppo[:], ones1[:, s0:s1],
                                 rowb_sb[:, RPB3:RPB3 + O],
                                 start=True, stop=False,
                                 skip_group_check=True)
                for k in range(HC):
                    _mm(nc, ppo[:], q2_sb[:, k, :], p3_sb[k],
                        start=False, stop=(k == HC - 1))
                pred_sb = smp.tile([ns, O], F32, tag="pred" + tag,
                                   name="predsb" + tag)
                nc.vector.tensor_copy(pred_sb[:], ppo[:])
                nc.sync.dma_start(out_d.ap()[s0:s1], pred_sb[:],
                                  single_packet=True)

            def post_chunk(d, prev_st):
                if d == 1:
                    # capture the pad vector v from the prepended pad token
                    # (chunk 0, column 0)
                    h2_t = prev_st[5]
                    for h in range(HC):
                        nc.vector.tensor_copy(acc_all[:, h, 0, 0:1],
                                              h2_t[:, h, 0:1])
                if d == CA - 1:
                    round_tr(0, RA, 0, CA)
                if d == CB - 1:
                    round_tr(32, RB, CA, CB)

            prev = None
            done = 0
            for ci in range(NC):
                cur = load_chunk(ci)
                for h in range(HC):
                    l1_h(cur, h)
                    if prev is not None:
                        l2_h(prev, h)
                if prev is not None:
                    done += 1
                    post_chunk(done, prev)
                prev = cur
            for h in range(HC):
                l2_h(prev, h)
            done += 1
            post_chunk(done, prev)

            # ---- epilogue: final transpose stage + predictor ----
            round_tr(64, RC, CB, NCOL)
            predictor(0, SLOTS, "b")

    nc.compile()
    return nc


def _layout_core(sl):
    """Place one core's tokens (already sorted by local slot `sl`), with a
    prepended pad token and pad insertion so no schedule chunk contains two
    valid-valid segment transitions.  Returns src index array (-1 = pad)
    into the core's local token order."""
    n = len(sl)
    seg_starts = [0] + (np.nonzero(np.diff(sl))[0] + 1).tolist() + [n]
    ends = chunk_ends(2 * (n + 4096) + 1024)

    def chunk_of(p):
        return bisect.bisect_right(ends, p)

    def end_of(p):
        return ends[chunk_of(p)]

    out = [-1]                      # prepended pad (v capture)
    pos = 1
    trans = {}
    for i in range(len(seg_starts) - 1):
        lo, hi = seg_starts[i], seg_starts[i + 1]
        if i > 0:
            at_start = pos == 0 or pos in ends
            if not at_start and trans.get(chunk_of(pos), 0) >= 1:
                pad = end_of(pos) - pos
                out.extend([-1] * pad)
                pos += pad
                at_start = True
            if not at_start:
                ch = chunk_of(pos)
                trans[ch] = trans.get(ch, 0) + 1
        out.extend(range(lo, hi))
        pos += hi - lo
    return np.array(out, dtype=np.int64)


def kernel(words, seg_ids, W1, b1, W2, b2, W3, b3,
           P1, pb1, P2, pb2, P3, pb3, batch_size, alpha_iter, **_):
    global _LAST_IN_MAPS
    words = np.asarray(words, dtype=np.float32)
    seg_ids = np.asarray(seg_ids).astype(np.int64)
    assert words.shape == (T, E), words.shape
    bs, ai = int(batch_size), int(alpha_iter)
    assert bs * ai == S

    # --- host-side index prep: cut the sorted token axis at segment
    # boundaries so each core owns whole segments ---
    counts = np.bincount(seg_ids, minlength=S)[:S]
    seg_off = np.concatenate([[0], np.cumsum(counts)])   # [S+1]
    # whole segments are assigned to cores by LPT bin-packing
    order = np.argsort(-counts, kind="stable")
    loads = np.zeros(N_CORES, dtype=np.int64)
    seg_list = [[] for _ in range(N_CORES)]
    for s in order:
        c = int(np.argmin(loads))
        seg_list[c].append(int(s))
        loads[c] += int(counts[s])
    seg_list = [sorted(a) for a in seg_list]
    slots_needed = max(len(a) for a in seg_list)
    SLOTS = min(128, max(MIN_SLOTS, ((slots_needed + 31) // 32) * 32))
    assert slots_needed <= SLOTS

    layouts = []
    core_sl = []
    core_tok = []
    for c in range(N_CORES):
        sl = np.concatenate(
            [np.full(counts[s], i, dtype=np.int64)
             for i, s in enumerate(seg_list[c])]) if seg_list[c] else \
            np.zeros(0, dtype=np.int64)
        tok = np.concatenate(
            [np.arange(seg_off[s], seg_off[s + 1], dtype=np.int64)
             for s in seg_list[c]]) if seg_list[c] else \
            np.zeros(0, dtype=np.int64)
        core_sl.append(sl)
        core_tok.append(tok)
        layouts.append(_layout_core(sl))
    maxlen = max(len(lm) for lm in layouts)
    t_sh = int(np.ceil(maxlen / 128) * 128)
    schedule = make_schedule(t_sh)
    NC = len(schedule)
    starts = np.concatenate([[0], np.cumsum(schedule)]).astype(int)
    CA = 9
    CB = NC - 1
    RA, RB, RC = 2 * CA, 2 * (CB - CA), 2 * (NC + 1 - CB)
    RT = 64 + RC
    RBM3 = 0
    RPB3 = RBM3 + H
    RCNT = RPB3 + O
    RPB1 = RCNT + SLOTS
    RPB2 = RPB1 + H
    RBCOLS = RPB2 + H

    xt = np.ascontiguousarray(words.T.astype(np.float16))    # [E, T] fp16

    key = ("nc", t_sh, SLOTS)
    if key not in _CACHE:
        _CACHE[key] = _build_nc(t_sh, SLOTS)
    nc = _CACHE[key]

    iota = np.broadcast_to(np.arange(TOK, dtype=np.float16),
                           (128, TOK)).copy()

    def pmajor(a, fi):
        # [fi, fo] -> [128, (fi//128) * fo] partition-major device layout
        fo = a.shape[1]
        return np.ascontiguousarray(
            a.reshape(fi // 128, 128, fo).transpose(1, 0, 2)
            .reshape(128, (fi // 128) * fo))

    W1h = np.asarray(W1, dtype=np.float16)
    W2h = np.asarray(W2, dtype=np.float16)
    w1p = pmajor(W1h, E).reshape(128, EC, HC, 128)       # [p, e, h, q]
    w1h0 = np.ascontiguousarray(w1p[:, :, 0, :]).reshape(128, EC * 128)
    w1r = np.ascontiguousarray(w1p[:, :, 1:, :]).reshape(
        128, EC * (HC - 1) * 128)
    w2p = pmajor(W2h, H).reshape(128, HC, HC, 128)       # [p, k, h, q]
    w2a = np.ascontiguousarray(w2p[:, 0:2]).reshape(128, 2 * HC * 128)
    w2b = np.ascontiguousarray(w2p[:, 2:4]).reshape(128, 2 * HC * 128)

    M = (np.asarray(W3, dtype=np.float32)
         @ np.asarray(P1, dtype=np.float32))              # [H, H] fused
    blob16 = np.concatenate([
        pmajor(M.astype(np.float16), H),                  # [128, HC*HC*128]
        pmajor(np.asarray(P2, dtype=np.float16), H),      # [128, HC*HC*128]
        pmajor(np.asarray(P3, dtype=np.float16), H),      # [128, HC*O]
    ], axis=1)
    bm3 = (np.asarray(P1, dtype=np.float32).T
           @ np.asarray(b3, dtype=np.float32).reshape(C))  # [H]

    b1p = np.asarray(b1, dtype=np.float32).reshape(HC, 128).T  # [128, HC]
    b2p = np.asarray(b2, dtype=np.float32).reshape(HC, 128).T

    common = {
        "w1h0": w1h0, "w1r": w1r, "w2a": w2a, "w2b": w2b,
        "blob16": blob16, "iota": iota,
    }

    in_maps = []
    perms = []
    for c in range(N_CORES):
        sl = core_sl[c]
        tok = core_tok[c]
        nseg = len(seg_list[c])
        lm = layouts[c]
        n = len(lm)
        assert n <= t_sh
        src = np.full(t_sh, -1, dtype=np.int64)
        src[:n] = lm
        valid = src >= 0
        xt_c = np.zeros((E, t_sh), dtype=np.float16)
        xt_c[:, valid] = xt[:, tok[src[valid]]]
        xt_dev = np.ascontiguousarray(
            xt_c.reshape(EC, 128, t_sh).transpose(1, 0, 2)
            .reshape(128, EC * t_sh))
        slot_of = np.full(t_sh, -1, dtype=np.int64)
        slot_of[valid] = sl[src[valid]]

        # positions (indices into seg_list[c]) that touch the stage-C
        # chunks run in the late predictor batch -> permute them to the
        # highest slot indices
        late = sorted(set(slot_of[starts[CB - 1]:].tolist()) - {-1})
        early = [p for p in range(nseg) if p not in late]
        assert len(late) <= S2, (c, late)
        assert len(early) <= SLOTS - S2, (c, len(early))
        perm = np.zeros(nseg, dtype=np.int64)
        for i, p in enumerate(early):
            perm[p] = i
        for i, p in enumerate(late):
            perm[p] = SLOTS - S2 + i

        bm1 = np.zeros(NC, dtype=np.float32)
        XA = np.zeros((NC, SLOTS), dtype=np.float32)
        XB = np.zeros((NC, SLOTS), dtype=np.float32)
        npad_of_slot = np.zeros(SLOTS, dtype=np.float32)
        for ci in range(NC):
            base, ct = int(starts[ci]), schedule[ci]
            ss = slot_of[base:base + ct]
            nz = np.nonzero((ss[1:] != ss[:-1]) & (ss[1:] >= 0)
                            & (ss[:-1] >= 0))[0]
            assert len(nz) <= 1, (c, ci, len(nz))
            vidx = np.nonzero(ss >= 0)[0]
            if len(vidx) == 0:
                continue
            nPa = int(vidx[0])                   # leading pads
            segA = int(perm[ss[vidx[0]]])
            if len(nz) == 1:                     # [P* A | B P*] chunk
                b = int(nz[0]) + 1
                segB = int(perm[ss[b]])
                nPb = int((ss[b:] == -1).sum())
                XA[ci, segA] += 1.0
                npad_of_slot[segA] += nPa
                XA[ci, segB] -= 1.0              # B gets full - head
                XB[ci, segB] += 1.0
                npad_of_slot[segB] += nPb
            else:                                # [P* A P*] chunk
                b = int(vidx[-1]) + 1
                XA[ci, segA] += 1.0
                npad_of_slot[segA] += nPa
            bm1[ci] = b

        blob32 = np.zeros((128, 2 * HC + NC), dtype=np.float32)
        blob32[:, 0:HC] = b1p
        blob32[:, HC:2 * HC] = b2p
        blob32[:, 2 * HC:] = np.broadcast_to(bm1, (128, NC))

        # coefficient stack rows, col-major plane-minor over acc columns
        # [v, chunk0..chunkNC-1], split into stages A|B|C
        Xall = np.zeros((NC + 1, 2, SLOTS), dtype=np.float32)
        Xall[0, 0] = -npad_of_slot                       # v column
        Xall[1:, 0] = XA
        Xall[1:, 1] = XB
        cfb = np.zeros((RT, SLOTS), dtype=np.float16)
        cfb[0:RA] = Xall[0:CA].reshape(RA, SLOTS)
        cfb[32:32 + RB] = Xall[CA:CB].reshape(RB, SLOTS)
        cfb[64:64 + RC] = Xall[CB:NC + 1].reshape(RC, SLOTS)

        rowb = np.zeros((1, RBCOLS), dtype=np.float16)
        rowb[0, RBM3:RBM3 + H] = bm3.astype(np.float16)
        rowb[0, RPB3:RPB3 + O] = \
            np.asarray(pb3, dtype=np.float16).reshape(O)
        cnt_c = np.zeros(SLOTS, dtype=np.float16)
        cnt_c[perm] = counts[seg_list[c]].astype(np.float16)
        rowb[0, RCNT:RCNT + SLOTS] = cnt_c
        rowb[0, RPB1:RPB1 + H] = np.asarray(pb1, dtype=np.float16)
        rowb[0, RPB2:RPB2 + H] = np.asarray(pb2, dtype=np.float16)

        perms.append(perm)
        in_maps.append({
            **common,
            "xt": xt_dev,
            "blob32": blob32,
            "cfb": cfb,
            "rowb": rowb,
        })

    _LAST_IN_MAPS = in_maps
    res = bass_utils.run_bass_kernel_spmd(nc, in_maps,
                                          core_ids=list(range(N_CORES)))
    pred = np.zeros((S, O), dtype=np.float32)
    for c in range(N_CORES):
        if seg_list[c]:
            pred[seg_list[c]] = res.results[c]["pred"][perms[c]]
    return pred.reshape(bs, ai, O).astype(np.float32)


# revision 80
# speedup vs baseline: 1.0062x; 1.0062x over previous
"""DeepSetPred Trainium2 kernel: 3-layer token encoder MLP + segment-sum +
predictor MLP on 8 NeuronCores, ZERO collectives.

Key structural wins over the naive mapping:

1. The third encoder layer is linear, so it commutes with the segment-sum:
       enc = segsum(h2 @ W3 + b3) = segsum(h2) @ W3 + counts * b3
   The per-token L3 matmul (25% of encoder FLOPs) and the one-hot
   segment-matmul collapse into one tiny [S,H]x[H,C] matmul after pooling.

2. segsum(h2) is computed without materializing anything token-major:
   - The host lays each core's (sorted) tokens out so every chunk contains
     at most ONE segment transition, inserting a few zero pad tokens when
     two boundaries would share a chunk.  One pad token is prepended so the
     pad vector v = tanh(W2'tanh(b1)+b2) can be captured on-device from
     chunk 0 (v corrections for pad-polluted sums are exact).
   - The L2 tanh on the ScalarEngine emits accum_out = full-chunk sum per
     H-partition (free side output).
   - One DVE scalar_tensor_tensor per (h-tile, chunk) computes the head
     sum via the fused (iota is_lt b) mult h2 -> accum_out form, with b
     per chunk as DATA, so the instruction stream is identical across all
     8 cores -> one NEFF.
   - hsegT[h, s] = sum_c head_c*XA[c,s] + full_c*XB[c,s] - npad*v, with
     the coefficient stack X as per-core input data, evaluated by tiny PE
     matmuls after transposing the accumulator.  The transpose+assembly
     runs in 3 stages; the first two are issued mid-loop (hidden under
     the token chunks), only the last 2 chunk columns assemble in the
     epilogue.

3. Everything stays fp16 on the matmul path (fp8 fails here: the ragged
   pooling amplifies per-token quantization noise by sqrt(count) ~ 32x).

4. Startup: the first chunks are small (512 tokens) and the startup-
   critical tensors lead 4 independent DMA rings (sync/gpsimd/scalar/
   vector) so the PE starts ~10us in (DMA pipe latency floor) and never
   starves.  Weights are packed into a few large blobs to cut DMA queue
   count (queue drains dominate NEFF teardown time).

5. Epilogue: predictor biases are applied via rank-1 PE matmuls so each
   tanh layer is ONE wide ACT over all 4 h-tiles; the whole predictor is
   a short pipelined chain of tiny matmuls.

Sharding: host cuts the sorted token axis at segment boundaries so every
segment belongs to exactly one core (no collectives); each core runs the
predictor on its own <=SLOTS segments and writes its slice of the output.
"""

import bisect

import numpy as np

import concourse.mybir as mybir
import concourse.tile as tile
from concourse import bacc
from concourse import bass_utils
from concourse.masks import make_identity

# Problem shapes (hardcoded per contract).
T, E, H, C, O = 131072, 256, 512, 256, 32
S = 128            # num segments
N_CORES = 8
TOK = 1024         # steady-state tokens per chunk
MIN_SLOTS = 32     # segments-per-core capacity
S2 = 16            # slots reserved for "late" segments (touch stage-C
                   # chunks); their predictor runs in the epilogue,
                   # everyone else's overlaps the token loop
F32 = mybir.dt.float32
F16 = mybir.dt.float16

EC = E // 128      # 2
HC = H // 128      # 4
CC = C // 128      # 2

_CACHE = {}
_LAST_IN_MAPS = None


def make_schedule(t_sh):
    """Chunk sizes: two 512s to fill the DMA pipeline fast, 1024s in the
    steady state, and a small-chunk tail so the final ACT/STT drain is
    short.  Only ever SUBDIVIDES the 512/512/1024... grid, so the host
    layout's <=1-transition-per-grid-interval guarantee carries over."""
    sched = []
    rem = t_sh
    for s in (512, 512):
        if rem <= 0:
            break
        c = min(s, rem)
        sched.append(c)
        rem -= c
    while rem > 1152:
        sched.append(TOK)
        rem -= TOK
    while rem > 640:
        sched.append(512)
        rem -= 512
    if rem > 128:
        sched.append(rem - 128)
        rem = 128
    if rem > 0:
        sched.append(rem)
    return sched


def chunk_ends(upto):
    """Ascending chunk END positions of the (t_sh-independent) grid."""
    ends = [512, 1024]
    while ends[-1] < upto:
        ends.append(ends[-1] + TOK)
    return ends


def make_groups(NC):
    """xt DMA groups: 4 single chunks then pairs (fewer DMA queues)."""
    g = []
    i = 0
    while i < NC:
        n = 1 if i < 4 else min(2, NC - i)
        g.append((i, n))
        i += n
    return g


def _mm(nc, out, lhsT, rhs, start, stop):
    nc.tensor.matmul(out, lhsT, rhs,
                     start=start, stop=stop, skip_group_check=True)


def _build_nc(t_sh, SLOTS):
    assert t_sh % 128 == 0
    schedule = make_schedule(t_sh)
    NC = len(schedule)
    CA = 9                       # stage-A accumulator columns (v + 8 chunks)
    assert NC >= CA + 3, NC
    NCOL = NC + 1                # acc columns: 0 = v, 1..NC = chunks
    CB = NC - 1                  # stage C covers the last two chunks
    RA, RB, RC = 2 * CA, 2 * (CB - CA), 2 * (NC + 1 - CB)
    assert CA < CB < NC and RA <= 32 and RB <= 32 and RC <= 32
    RT = 64 + RC                 # accT rows; stages at partitions 0/32/64
    # rowblob column offsets (must match host); all operands live in ONE
    # SBUF row so every matmul slice has base partition 0
    RBM3 = 0
    RPB3 = RBM3 + H
    RCNT = RPB3 + O
    RPB1 = RCNT + SLOTS
    RPB2 = RPB1 + H
    RBCOLS = RPB2 + H

    nc = bacc.Bacc("TRN2", target_bir_lowering=False, debug=False,
                   num_devices=N_CORES)

    xt_d = nc.dram_tensor("xt", [128, EC * t_sh], F16, kind="ExternalInput")
    w1h0_d = nc.dram_tensor("w1h0", [128, EC * 128], F16,
                            kind="ExternalInput")
    w1r_d = nc.dram_tensor("w1r", [128, EC * (HC - 1) * 128], F16,
                           kind="ExternalInput")
    w2a_d = nc.dram_tensor("w2a", [128, 2 * HC * 128], F16,
                           kind="ExternalInput")
    w2b_d = nc.dram_tensor("w2b", [128, 2 * HC * 128], F16,
                           kind="ExternalInput")
    # blob16: [M=W3@P1 (fused, skips the enc stage) | p2 | p3] layout
    B16M = 0
    B16P2 = B16M + HC * HC * 128
    B16P3 = B16P2 + HC * HC * 128
    B16COLS = B16P3 + HC * O
    blob16_d = nc.dram_tensor("blob16", [128, B16COLS], F16,
                              kind="ExternalInput")
    # blob32: [b1 (HC) | b2 (HC) | bm1 (NC)]
    blob32_d = nc.dram_tensor("blob32", [128, 2 * HC + NC], F32,
                              kind="ExternalInput")
    cfb_d = nc.dram_tensor("cfb", [RT, SLOTS], F16, kind="ExternalInput")
    rowb_d = nc.dram_tensor("rowb", [1, RBCOLS], F16, kind="ExternalInput")
    iota_d = nc.dram_tensor("iota", [128, TOK], F16, kind="ExternalInput")
    out_d = nc.dram_tensor("pred", [SLOTS, O], F32, kind="ExternalOutput")

    LT = mybir.AluOpType.is_lt
    MUL = mybir.AluOpType.mult
    TANH = mybir.ActivationFunctionType.Tanh

    groups = make_groups(NC)
    chunk_group = {}
    starts = np.concatenate([[0], np.cumsum(schedule)]).astype(int)
    for gi, (c0, n) in enumerate(groups):
        for k in range(n):
            ci = c0 + k
            chunk_group[ci] = (gi, int(starts[ci] - starts[c0]))

    with tile.TileContext(nc) as tc:
        with tc.tile_pool(name="wts", bufs=1) as wp, \
             tc.tile_pool(name="xt", bufs=4) as xtp, \
             tc.tile_pool(name="h1", bufs=2) as h1p, \
             tc.tile_pool(name="h2", bufs=3) as h2p, \
             tc.tile_pool(name="scr", bufs=3) as scp, \
             tc.tile_pool(name="small", bufs=1) as smp, \
             tc.tile_pool(name="ps1", bufs=2, space="PSUM") as ps1p, \
             tc.tile_pool(name="ps2", bufs=2, space="PSUM") as ps2p:

            # warm the ACT tanh table before the scalar queue fills
            warm_sb = smp.tile([1, 1], F32, tag="warm", name="warm")
            nc.gpsimd.memset(warm_sb[:], 0.0)
            warm_o = smp.tile([1, 1], F32, tag="warmo", name="warmo")
            nc.scalar.activation(warm_o[:], warm_sb[:], TANH)

            # ---- startup-critical DMAs on 3 rings, in priority order.
            # Chunk 0 rides the sync ring ALONE so the first matmuls gate
            # on exactly two queues (sync: c0, scalar: w1h0); weights are
            # split scalar (w1) / gpsimd (biases + w2 + iota) so each
            # queue's serial delivery matches when the pipeline needs it.
            w1h0_t = wp.tile([128, EC, 128], F16, tag="w1h0", name="w1h0")
            nc.scalar.dma_start(
                w1h0_t[:], w1h0_d.ap().rearrange("p (e q) -> p e q", e=EC))
            w1r_t = wp.tile([128, EC, HC - 1, 128], F16, tag="w1r",
                            name="w1r")
            nc.scalar.dma_start(
                w1r_t[:], w1r_d.ap().rearrange("p (e h q) -> p e h q",
                                               e=EC, q=128))
            cfb_sb = smp.tile([RT, SLOTS], F16, tag="cfb", name="cfb")
            nc.scalar.dma_start(cfb_sb[:], cfb_d.ap())
            rowb_sb = smp.tile([1, RBCOLS], F16, tag="rowb", name="rowb")
            nc.scalar.dma_start(rowb_sb[:], rowb_d.ap())
            blob32_sb = smp.tile([128, 2 * HC + NC], F32, tag="b32",
                                 name="b32")
            nc.gpsimd.dma_start(blob32_sb[:], blob32_d.ap())
            w2a_t = wp.tile([128, 2, HC, 128], F16, tag="w2a", name="w2a")
            nc.gpsimd.dma_start(
                w2a_t[:], w2a_d.ap().rearrange("p (k h q) -> p k h q",
                                               k=2, q=128))
            w2b_t = wp.tile([128, 2, HC, 128], F16, tag="w2b", name="w2b")
            nc.gpsimd.dma_start(
                w2b_t[:],
                w2b_d.ap().rearrange("p (k h q) -> p k h q", k=2, q=128))
            iota_sb = smp.tile([128, TOK], F16, tag="iota", name="iota")
            nc.gpsimd.dma_start(iota_sb[:], iota_d.ap())
            # epilogue blob: not needed until ~190us; held back so it
            # cannot compete with the startup crunch for HBM bandwidth
            w3p_t = wp.tile([128, B16COLS], F16, tag="b16", name="b16")

            w1_sb = [[(w1h0_t[:, e, :] if h == 0 else w1r_t[:, e, h - 1, :])
                      for h in range(HC)] for e in range(EC)]
            w2_sb = [[(w2a_t[:, k, h, :] if k < 2 else w2b_t[:, k - 2, h, :])
                      for h in range(HC)] for k in range(HC)]
            m_sb = [[w3p_t[:, B16M + (k * HC + h) * 128:
                            B16M + (k * HC + h + 1) * 128]
                      for h in range(HC)] for k in range(HC)]
            p2_sb = [[w3p_t[:, B16P2 + (k * HC + h) * 128:
                            B16P2 + (k * HC + h + 1) * 128]
                      for h in range(HC)] for k in range(HC)]
            p3_sb = [w3p_t[:, B16P3 + k * O:B16P3 + (k + 1) * O]
                     for k in range(HC)]
            b1_sb = blob32_sb[:, 0:HC]
            b2_sb = blob32_sb[:, HC:2 * HC]
            bm1_sb = blob32_sb[:, 2 * HC:2 * HC + NC]

            # ---- xt group tiles: issue every group DMA up front on the
            # sync ring; pool semaphores throttle execution ----
            gmax = max(int(starts[c0 + n] - starts[c0])
                       for (c0, n) in groups)
            gtiles = []
            for gi, (c0, n) in enumerate(groups):
                glen = int(starts[c0 + n] - starts[c0])
                gt = xtp.tile([128, EC, glen], F16, tag="xt", name="xt",
                              padded_shape=[128, EC, gmax])
                src = xt_d.ap().rearrange("p (e t) -> p e t", e=EC)
                gb = int(starts[c0])
                # stagger the early big-chunk fetches so they don't steal
                # HBM bandwidth from the critical weight loads
                delay = {2: 0.0115, 3: 0.013}.get(gi)
                with tc.tile_wait_until(delay, enable=delay is not None):
                    if gi in (1, 2):
                        # early chunks in halves: their first matmul
                        # pieces start ~1us sooner, keeping the PE clock
                        # ramp unbroken through the startup crunch
                        half = glen // 2
                        nc.sync.dma_start(gt[:, :, 0:half],
                                          src[:, :, gb:gb + half])
                        nc.sync.dma_start(gt[:, :, half:glen],
                                          src[:, :, gb + half:gb + glen])
                    else:
                        nc.sync.dma_start(gt[:], src[:, :, gb:gb + glen])
                if gi == 4:
                    with tc.tile_wait_until(0.025):
                        nc.sync.dma_start(w3p_t[:], blob16_d.ap())
                gtiles.append(gt)

            ones1 = smp.tile([1, SLOTS], F16, tag="ones1", name="ones1")
            nc.gpsimd.memset(ones1[:], 1.0)
            ident = smp.tile([128, 128], F32, tag="ident", name="ident")
            make_identity(nc, ident[:])

            # pooling accumulators: [h, col, plane]; col 0 plane 0 = v,
            # cols 1..NC = chunks; plane 0 = head sum, plane 1 = full sum
            acc_all = smp.tile([128, HC, NCOL, 2], F32, tag="acc",
                               name="acc")
            nc.gpsimd.memset(acc_all[:], 0.0)

            def pieces(ci, ct):
                if ci == 1 and ct == 512:
                    return [(0, 256), (256, 256)]
                if ct <= 512:
                    return [(0, ct)]
                return [(0, 512), (512, ct - 512)]

            def load_chunk(ci):
                ct = schedule[ci]
                gi, off = chunk_group[ci]
                h1_t = h1p.tile([128, HC, ct], F16, tag="h1", name="h1",
                                padded_shape=[128, HC, TOK])
                h2_t = h2p.tile([128, HC, ct], F16, tag="h2", name="h2",
                                padded_shape=[128, HC, TOK])
                return (ci, ct, gtiles[gi], off, h1_t, h2_t)

            def l1_h(st, h):
                ci, ct, xt_t, off, h1_t, _ = st
                ps1 = ps1p.tile([128, ct], F32, tag="ps1", name="ps1",
                                padded_shape=[128, TOK])
                for (hb, hl) in pieces(ci, ct):
                    for e in range(EC):
                        _mm(nc, ps1[:, hb:hb + hl], w1_sb[e][h],
                            xt_t[:, e, off + hb:off + hb + hl],
                            start=(e == 0), stop=(e == EC - 1))
                nc.scalar.activation(h1_t[:, h, :], ps1[:], TANH,
                                     bias=b1_sb[:, h:h + 1])

            def l2_h(st, h):
                ci, ct, _, _, h1_t, h2_t = st
                # final chunk: no L1 work remains, so borrow ps1p for the
                # odd h-groups -- all 4 L2 groups get their own PSUM buffer
                # and drain through ACT/STT in parallel instead of
                # serializing on ps2p's two buffers
                if ci == NC - 1 and h % 2 == 1:
                    ps2 = ps1p.tile([128, ct], F32, tag="ps1", name="ps2f",
                                    padded_shape=[128, TOK])
                else:
                    ps2 = ps2p.tile([128, ct], F32, tag="ps2", name="ps2",
                                    padded_shape=[128, TOK])
                for (hb, hl) in pieces(ci, ct):
                    for k in range(HC):
                        _mm(nc, ps2[:, hb:hb + hl], w2_sb[k][h],
                            h1_t[:, k, hb:hb + hl],
                            start=(k == 0), stop=(k == HC - 1))
                nc.scalar.activation(h2_t[:, h, :], ps2[:], TANH,
                                     bias=b2_sb[:, h:h + 1],
                                     accum_out=acc_all[:, h, ci + 1, 1:2])
                # head sum: (iota < b) * h2 summed along tokens, b is data
                sc = scp.tile([128, ct], F16, tag="scr", name="scr",
                              padded_shape=[128, TOK])
                nc.vector.scalar_tensor_tensor(
                    sc[:], iota_sb[:, :ct], bm1_sb[:, ci:ci + 1],
                    h2_t[:, h, :], LT, MUL,
                    accum_out=acc_all[:, h, ci + 1, 0:1])

            # staged accumulator assembly: transpose columns as soon as
            # their chunks are done, overlapped with the token loop.  All
            # stages land in ONE stack (at partition offsets 0/32/64 --
            # engine accesses must start at a multiple of 32) so hsegT
            # needs one matmul per h; the gap rows multiply by zero
            # coefficients and are memset once so they can't be NaN.
            accT = smp.tile([RT, HC, 128], F16, tag="accT", name="accT")
            nc.gpsimd.memset(accT[:], 0.0)

            def round_tr(r0, rows, c0, c1):
                for h in range(HC):
                    tp = ps1p.tile([rows, 128], F32, tag="ps1", name="tp")
                    nc.tensor.transpose(tp[:], acc_all[:, h, c0:c1, :],
                                        ident[:])
                    nc.vector.tensor_copy(accT[r0:r0 + rows, h, :], tp[:])

            # ---- predictor MLP over a slot range; biases via rank-1
            # matmuls so each tanh layer is ONE wide ACT over all h tiles.
            # Runs twice: slots [0:S1) ("early" segments, no stage-C
            # coefficients) are issued right after round B so they overlap
            # the last token chunks; slots [S1:SLOTS) run in the epilogue.
            def predictor(s0, s1, tag):
                ns = s1 - s0
                hs_ps = ps2p.tile([128, HC, ns], F32, tag="ps2",
                                  name="hs" + tag)
                for h in range(HC):
                    _mm(nc, hs_ps[:, h, :], accT[:, h, :],
                        cfb_sb[:, s0:s1], start=True, stop=True)
                hsegT = smp.tile([128, HC, ns], F16, tag="hsegT" + tag,
                                 name="hsegT" + tag)
                nc.vector.tensor_copy(hsegT[:], hs_ps[:])

                # q1_pre = (W3 P1)^T hseg + (P1^T b3) x cnt + pb1 x 1
                # (W3@P1 fused on the host -> the enc stage disappears)
                q1_ps = ps2p.tile([128, HC, ns], F32, tag="ps2",
                                  name="q1p" + tag)
                for h in range(HC):
                    nc.tensor.matmul(q1_ps[:, h, :],
                                     rowb_sb[:, RBM3 + h * 128:
                                             RBM3 + (h + 1) * 128],
                                     rowb_sb[:, RCNT + s0:RCNT + s1],
                                     start=True, stop=False,
                                     skip_group_check=True)
                    nc.tensor.matmul(q1_ps[:, h, :],
                                     rowb_sb[:, RPB1 + h * 128:
                                             RPB1 + (h + 1) * 128],
                                     ones1[:, s0:s1], start=False,
                                     stop=False, skip_group_check=True)
                    for k in range(HC):
                        _mm(nc, q1_ps[:, h, :], m_sb[k][h],
                            hsegT[:, k, :], start=False,
                            stop=(k == HC - 1))
                q1_sb = smp.tile([128, HC, ns], F16, tag="q1" + tag,
                                 name="q1" + tag)
                nc.scalar.activation(q1_sb[:], q1_ps[:], TANH)

                q2_ps = ps1p.tile([128, HC, ns], F32, tag="ps1",
                                  name="q2p" + tag)
                for h in range(HC):
                    nc.tensor.matmul(q2_ps[:, h, :],
                                     rowb_sb[:, RPB2 + h * 128:
                                             RPB2 + (h + 1) * 128],
                                     ones1[:, s0:s1], start=True,
                                     stop=False, skip_group_check=True)
                    for k in range(HC):
                        _mm(nc, q2_ps[:, h, :], p2_sb[k][h],
                            q1_sb[:, k, :], start=False,
                            stop=(k == HC - 1))
                q2_sb = smp.tile([128, HC, ns], F16, tag="q2" + tag,
                                 name="q2" + tag)
                nc.scalar.activation(q2_sb[:], q2_ps[:], TANH)

                # final: pred[slot, o] = q2.T @ P3 + pb3
                ppo = ps2p.tile([ns, O], F32, tag="ps2", name="ppo" + tag)
                nc.tensor.matmul(ppo[:], ones1[:, s0:s1],
                                 rowb_sb[:, RPB3:RPB3 + O],
                                 start=True, stop=False,
                                 skip_group_check=True)
                for k in range(HC):
                    _mm(nc, ppo[:], q2_sb[:, k, :], p3_sb[k],
                        start=False, stop=(k == HC - 1))
                pred_sb = smp.tile([ns, O], F32, tag="pred" + tag,
                                   name="predsb" + tag)
                nc.vector.tensor_copy(pred_sb[:], ppo[:])
                nc.sync.dma_start(out_d.ap()[s0:s1], pred_sb[:],
                                  single_packet=True)

            def post_chunk(d, prev_st):
                if d == 1:
                    # capture the pad vector v from the prepended pad token
                    # (chunk 0, column 0)
                    h2_t = prev_st[5]
                    for h in range(HC):
                        nc.vector.tensor_copy(acc_all[:, h, 0, 0:1],
                                              h2_t[:, h, 0:1])
                if d == CA - 1:
                    round_tr(0, RA, 0, CA)
                if d == CB - 1:
                    round_tr(32, RB, CA, CB)

            prev = None
            done = 0
            for ci in range(NC):
                cur = load_chunk(ci)
                for h in range(HC):
                    l1_h(cur, h)
                    if prev is not None:
                        l2_h(prev, h)
                if prev is not None:
                    done += 1
                    post_chunk(done, prev)
                prev = cur
            for h in range(HC):
                l2_h(prev, h)
            done += 1
            post_chunk(done, prev)

            # ---- epilogue: final transpose stage + predictor ----
            round_tr(64, RC, CB, NCOL)
            predictor(0, SLOTS, "b")

    nc.compile()
    return nc


def _layout_core(sl):
    """Place one core's tokens (already sorted by local slot `sl`), with a
    prepended pad token and pad insertion so no schedule chunk contains two
    valid-valid segment transitions.  Returns src index array (-1 = pad)
    into the core's local token order."""
    n = len(sl)
    seg_starts = [0] + (np.nonzero(np.diff(sl))[0] + 1).tolist() + [n]
    ends = chunk_ends(2 * (n + 4096) + 1024)

    def chunk_of(p):
        return bisect.bisect_right(ends, p)

    def end_of(p):
        return ends[chunk_of(p)]

    out = [-1]                      # prepended pad (v capture)
    pos = 1
    trans = {}
    for i in range(len(seg_starts) - 1):
        lo, hi = seg_starts[i], seg_starts[i + 1]
        if i > 0:
            at_start = pos == 0 or pos in ends
            if not at_start and trans.get(chunk_of(pos), 0) >= 1:
                pad = end_of(pos) - pos
                out.extend([-1] * pad)
                pos += pad
                at_start = True
            if not at_start:
                ch = chunk_of(pos)
                trans[ch] = trans.get(ch, 0) + 1
        out.extend(range(lo, hi))
        pos += hi - lo
    return np.array(out, dtype=np.int64)


def kernel(words, seg_ids, W1, b1, W2, b2, W3, b3,
           P1, pb1, P2, pb2, P3, pb3, batch_size, alpha_iter, **_):
    global _LAST_IN_MAPS
    words = np.asarray(words, dtype=np.float32)
    seg_ids = np.asarray(seg_ids).astype(np.int64)
    assert words.shape == (T, E), words.shape
    bs, ai = int(batch_size), int(alpha_iter)
    assert bs * ai == S

    # --- host-side index prep: cut the sorted token axis at segment
    # boundaries so each core owns whole segments ---
    counts = np.bincount(seg_ids, minlength=S)[:S]
    seg_off = np.concatenate([[0], np.cumsum(counts)])   # [S+1]
    # whole segments are assigned to cores by LPT bin-packing
    order = np.argsort(-counts, kind="stable")
    loads = np.zeros(N_CORES, dtype=np.int64)
    seg_list = [[] for _ in range(N_CORES)]
    for s in order:
        c = int(np.argmin(loads))
        seg_list[c].append(int(s))
        loads[c] += int(counts[s])
    seg_list = [sorted(a) for a in seg_list]
    slots_needed = max(len(a) for a in seg_list)
    SLOTS = min(128, max(MIN_SLOTS, ((slots_needed + 31) // 32) * 32))
    assert slots_needed <= SLOTS

    layouts = []
    core_sl = []
    core_tok = []
    for c in range(N_CORES):
        sl = np.concatenate(
            [np.full(counts[s], i, dtype=np.int64)
             for i, s in enumerate(seg_list[c])]) if seg_list[c] else \
            np.zeros(0, dtype=np.int64)
        tok = np.concatenate(
            [np.arange(seg_off[s], seg_off[s + 1], dtype=np.int64)
             for s in seg_list[c]]) if seg_list[c] else \
            np.zeros(0, dtype=np.int64)
        core_sl.append(sl)
        core_tok.append(tok)
        layouts.append(_layout_core(sl))
    maxlen = max(len(lm) for lm in layouts)
    t_sh = int(np.ceil(maxlen / 128) * 128)
    schedule = make_schedule(t_sh)
    NC = len(schedule)
    starts = np.concatenate([[0], np.cumsum(schedule)]).astype(int)
    CA = 9
    CB = NC - 1
    RA, RB, RC = 2 * CA, 2 * (CB - CA), 2 * (NC + 1 - CB)
    RT = 64 + RC
    RBM3 = 0
    RPB3 = RBM3 + H
    RCNT = RPB3 + O
    RPB1 = RCNT + SLOTS
    RPB2 = RPB1 + H
    RBCOLS = RPB2 + H

    xt = np.ascontiguousarray(words.T.astype(np.float16))    # [E, T] fp16

    key = ("nc", t_sh, SLOTS)
    if key not in _CACHE:
        _CACHE[key] = _build_nc(t_sh, SLOTS)
    nc = _CACHE[key]

    iota = np.broadcast_to(np.arange(TOK, dtype=np.float16),
                           (128, TOK)).copy()

    def pmajor(a, fi):
        # [fi, fo] -> [128, (fi//128) * fo] partition-major device layout
        fo = a.shape[1]
        return np.ascontiguousarray(
            a.reshape(fi // 128, 128, fo).transpose(1, 0, 2)
            .reshape(128, (fi // 128) * fo))

    W1h = np.asarray(W1, dtype=np.float16)
    W2h = np.asarray(W2, dtype=np.float16)
    w1p = pmajor(W1h, E).reshape(128, EC, HC, 128)       # [p, e, h, q]
    w1h0 = np.ascontiguousarray(w1p[:, :, 0, :]).reshape(128, EC * 128)
    w1r = np.ascontiguousarray(w1p[:, :, 1:, :]).reshape(
        128, EC * (HC - 1) * 128)
    w2p = pmajor(W2h, H).reshape(128, HC, HC, 128)       # [p, k, h, q]
    w2a = np.ascontiguousarray(w2p[:, 0:2]).reshape(128, 2 * HC * 128)
    w2b = np.ascontiguousarray(w2p[:, 2:4]).reshape(128, 2 * HC * 128)

    M = (np.asarray(W3, dtype=np.float32)
         @ np.asarray(P1, dtype=np.float32))              # [H, H] fused
    blob16 = np.concatenate([
        pmajor(M.astype(np.float16), H),                  # [128, HC*HC*128]
        pmajor(np.asarray(P2, dtype=np.float16), H),      # [128, HC*HC*128]
        pmajor(np.asarray(P3, dtype=np.float16), H),      # [128, HC*O]
    ], axis=1)
    bm3 = (np.asarray(P1, dtype=np.float32).T
           @ np.asarray(b3, dtype=np.float32).reshape(C))  # [H]

    b1p = np.asarray(b1, dtype=np.float32).reshape(HC, 128).T  # [128, HC]
    b2p = np.asarray(b2, dtype=np.float32).reshape(HC, 128).T

    common = {
        "w1h0": w1h0, "w1r": w1r, "w2a": w2a, "w2b": w2b,
        "blob16": blob16, "iota": iota,
    }

    in_maps = []
    perms = []
    for c in range(N_CORES):
        sl = core_sl[c]
        tok = core_tok[c]
        nseg = len(seg_list[c])
        lm = layouts[c]
        n = len(lm)
        assert n <= t_sh
        src = np.full(t_sh, -1, dtype=np.int64)
        src[:n] = lm
        valid = src >= 0
        xt_c = np.zeros((E, t_sh), dtype=np.float16)
        xt_c[:, valid] = xt[:, tok[src[valid]]]
        xt_dev = np.ascontiguousarray(
            xt_c.reshape(EC, 128, t_sh).transpose(1, 0, 2)
            .reshape(128, EC * t_sh))
        slot_of = np.full(t_sh, -1, dtype=np.int64)
        slot_of[valid] = sl[src[valid]]

        # positions (indices into seg_list[c]) that touch the stage-C
        # chunks run in the late predictor batch -> permute them to the
        # highest slot indices
        late = sorted(set(slot_of[starts[CB - 1]:].tolist()) - {-1})
        early = [p for p in range(nseg) if p not in late]
        assert len(late) <= S2, (c, late)
        assert len(early) <= SLOTS - S2, (c, len(early))
        perm = np.zeros(nseg, dtype=np.int64)
        for i, p in enumerate(early):
            perm[p] = i
        for i, p in enumerate(late):
            perm[p] = SLOTS - S2 + i

        bm1 = np.zeros(NC, dtype=np.float32)
        XA = np.zeros((NC, SLOTS), dtype=np.float32)
        XB = np.zeros((NC, SLOTS), dtype=np.float32)
        npad_of_slot = np.zeros(SLOTS, dtype=np.float32)
        for ci in range(NC):
            base, ct = int(starts[ci]), schedule[ci]
            ss = slot_of[base:base + ct]
            nz = np.nonzero((ss[1:] != ss[:-1]) & (ss[1:] >= 0)
                            & (ss[:-1] >= 0))[0]
            assert len(nz) <= 1, (c, ci, len(nz))
            vidx = np.nonzero(ss >= 0)[0]
            if len(vidx) == 0:
                continue
            nPa = int(vidx[0])                   # leading pads
            segA = int(perm[ss[vidx[0]]])
            if len(nz) == 1:                     # [P* A | B P*] chunk
                b = int(nz[0]) + 1
                segB = int(perm[ss[b]])
                nPb = int((ss[b:] == -1).sum())
                XA[ci, segA] += 1.0
                npad_of_slot[segA] += nPa
                XA[ci, segB] -= 1.0              # B gets full - head
                XB[ci, segB] += 1.0
                npad_of_slot[segB] += nPb
            else:                                # [P* A P*] chunk
                b = int(vidx[-1]) + 1
                XA[ci, segA] += 1.0
                npad_of_slot[segA] += nPa
            bm1[ci] = b

        blob32 = np.zeros((128, 2 * HC + NC), dtype=np.float32)
        blob32[:, 0:HC] = b1p
        blob32[:, HC:2 * HC] = b2p
        blob32[:, 2 * HC:] = np.broadcast_to(bm1, (128, NC))

        # coefficient stack rows, col-major plane-minor over acc columns
        # [v, chunk0..chunkNC-1], split into stages A|B|C
        Xall = np.zeros((NC + 1, 2, SLOTS), dtype=np.float32)
        Xall[0, 0] = -npad_of_slot                       # v column
        Xall[1:, 0] = XA
        Xall[1:, 1] = XB
        cfb = np.zeros((RT, SLOTS), dtype=np.float16)
        cfb[0:RA] = Xall[0:CA].reshape(RA, SLOTS)
        cfb[32:32 + RB] = Xall[CA:CB].reshape(RB, SLOTS)
        cfb[64:64 + RC] = Xall[CB:NC + 1].reshape(RC, SLOTS)

        rowb = np.zeros((1, RBCOLS), dtype=np.float16)
        rowb[0, RBM3:RBM3 + H] = bm3.astype(np.float16)
        rowb[0, RPB3:RPB3 + O] = \
            np.asarray(pb3, dtype=np.float16).reshape(O)
        cnt_c = np.zeros(SLOTS, dtype=np.float16)
        cnt_c[perm] = counts[seg_list[c]].astype(np.float16)
        rowb[0, RCNT:RCNT + SLOTS] = cnt_c
        rowb[0, RPB1:RPB1 + H] = np.asarray(pb1, dtype=np.float16)
        rowb[0, RPB2:RPB2 + H] = np.asarray(pb2, dtype=np.float16)

        perms.append(perm)
        in_maps.append({
            **common,
            "xt": xt_dev,
            "blob32": blob32,
            "cfb": cfb,
            "rowb": rowb,
        })

    _LAST_IN_MAPS = in_maps
    res = bass_utils.run_bass_kernel_spmd(nc, in_maps,
                                          core_ids=list(range(N_CORES)))
    pred = np.zeros((S, O), dtype=np.float32)
    for c in range(N_CORES):
        if seg_list[c]:
            pred[seg_list[c]] = res.results[c]["pred"][perms[c]]
    return pred.reshape(bs, ai, O).astype(np.float32)
